# revision 2
# baseline (speedup 1.0000x reference)
"""HGCN (2-layer hyperbolic GCN) on 8 trn2 NeuronCores.

Sharding: nodes (and the segment_sum output) are sharded across the 8 cores
by node id; edges are partitioned by destination shard so each core
scatter-adds locally; the tangent-space features t are all-gathered (bf16)
once per layer; the small [128,128] weights are replicated.

Per-core per-layer pipeline:
  stage1  dense hyperbolic linear (hyp_linear + logmap0).  All elementwise
          hyperbolic math reduces to per-row scalars because every tensor in
          the chain is a linear combination of mx = a @ W.T and the constant
          bias vector hb; norms/dots come from ||a||^2, ||mx||^2, <mx,hb>.
  AG      AllGather of t (bf16) -> t_full in shared DRAM.
  stage2  per dst-block of 128 nodes: indirect-DMA gather of the block's
          (padded) edges' source rows, one-hot bf16 matmuls accumulate the
          segment sum in PSUM, add the self-loop t, then the output
          hyperbolic transform (expmap0 + bias) again via per-row scalars.
"""

import sys

sys.path.insert(0, "/opt/trn_rl_repo")

import numpy as np

import concourse.bass as bass
import concourse.bacc as bacc
import concourse.tile as tile
from concourse import mybir
from concourse.bass import IndirectOffsetOnAxis
from concourse.bass_utils import run_bass_kernel_spmd
from concourse.masks import make_identity
from concourse.tile import TileContext

F32 = mybir.dt.float32
BF16 = mybir.dt.bfloat16
I32 = mybir.dt.int32
AL = mybir.AluOpType
AF = mybir.ActivationFunctionType

P = 128          # partitions / feature dim
N = 100000       # nodes
D = 128          # feature dim
E = 800000       # edges
C = 8            # cores
NL = N // C      # 12500 nodes per core
NT = (NL + P - 1) // P   # 98 row tiles per core
NLP = NT * P     # 12544 padded rows per core
NGP = NLP * C    # 100352 padded global rows

MIN_NORM = 1e-15
CLIP = 1.0 - 1e-7
MAXN = 1.0 - 4e-3   # proj maxnorm for c=1

import os
DEBUG_STAGE = os.environ.get("KDEBUG", "")

GCHUNKS = 48     # chunks (of 128 edges) per dma_gather call
NSEG = 4         # t_full segments (int16-addressable row ranges)
SEGR = NGP // NSEG


# ----------------------------------------------------------------------------
# host-side hyperbolic helpers (c = 1)
# ----------------------------------------------------------------------------
def _np_norm(x):
    return np.maximum(np.linalg.norm(x, axis=-1, keepdims=True), MIN_NORM)


def _np_proj(x):
    n = _np_norm(x)
    return np.where(n > MAXN, x / n * MAXN, x)


def _np_expmap0(u):
    un = _np_norm(u)
    return np.tanh(un) * u / un


def _np_hb(b):
    return _np_proj(_np_expmap0(b[None, :].astype(np.float64)))[0].astype(np.float32)


# ----------------------------------------------------------------------------
# bass kernel builder
# ----------------------------------------------------------------------------
def build_kernel(sched, y2_lin, y2_post):
    """sched: chunk stream of (block, segment, kk) — identical on all cores.
    y2_lin/y2_post: per-layer ||hb||^2 consts (python floats), lists len 2."""
    TC = len(sched)

    nc = bacc.Bacc("TRN2", num_devices=C)

    xp = nc.dram_tensor("xp", [P, NT * D], F32, kind="ExternalInput")
    w_t = [nc.dram_tensor(f"w{l}t", [P, D], F32, kind="ExternalInput") for l in (1, 2)]
    hbl = [nc.dram_tensor(f"hbl{l}", [P, D], F32, kind="ExternalInput") for l in (1, 2)]
    hbp = [nc.dram_tensor(f"hbp{l}", [P, D], F32, kind="ExternalInput") for l in (1, 2)]
    iota_in = nc.dram_tensor("iota", [P, D], BF16, kind="ExternalInput")
    idxg = nc.dram_tensor("idxg", [P, TC * 8], mybir.dt.int16, kind="ExternalInput")
    dstg = nc.dram_tensor("dstg", [P, TC], BF16, kind="ExternalInput")
    outp = nc.dram_tensor("outp", [P, NT * D], F32, kind="ExternalOutput")

    tsrc = [nc.dram_tensor(f"tsrc{l}", [NLP, D], BF16, kind="Internal") for l in (1, 2)]
    tful = [
        nc.dram_tensor(f"tful{l}", [NGP, D], BF16, kind="Internal", addr_space="Shared")
        for l in (1, 2)
    ]
    rg = [list(range(C))]

    with TileContext(nc) as tc:
        with (
            tc.tile_pool(name="const", bufs=1) as cpool,
            tc.tile_pool(name="big", bufs=1) as bpool,
            tc.tile_pool(name="cols", bufs=1) as colp,
            tc.tile_pool(name="scr", bufs=4) as spool,
            tc.tile_pool(name="aT", bufs=3) as atp,
            tc.tile_pool(name="uv", bufs=3) as uvp,
            tc.tile_pool(name="gat", bufs=2) as gpool,
            tc.tile_pool(name="oh", bufs=2) as ohpool,
            tc.tile_pool(name="psT", bufs=2, space="PSUM") as psT,
            tc.tile_pool(name="psM", bufs=2, space="PSUM") as psM,
            tc.tile_pool(name="psA", bufs=3, space="PSUM") as psA,
        ):
            # ---- constants into SBUF
            def load_const(dram, dt):
                t = cpool.tile(list(dram.shape), dt, name=dram.name + "_sb")
                nc.sync.dma_start(t[:], dram[:])
                return t

            w_sb = [load_const(w, F32) for w in w_t]
            hbl_sb = [load_const(h, F32) for h in hbl]
            hbp_sb = [load_const(h, F32) for h in hbp]
            iota_sb = load_const(iota_in, BF16)
            idxg_sb = load_const(idxg, mybir.dt.int16)
            dstg_sb = load_const(dstg, BF16)
            ident = cpool.tile([P, P], F32)
            make_identity(nc, ident[:])

            a_buf = bpool.tile([P, NT * D], F32)
            tbf = bpool.tile([P, NT * D], BF16)

            nc.sync.dma_start(a_buf[:], xp[:])

            def a_t(b):
                return a_buf[:, b * D : (b + 1) * D]

            def t_t(b):
                return tbf[:, b * D : (b + 1) * D]

            # ---- batched per-row scalar helpers ([P, NT] tiles) ----
            def col():
                return colp.tile([P, NT], F32, tag="col", name="col", bufs=48)

            def tt(in0, in1, op):
                o = col()
                nc.vector.tensor_tensor(out=o[:], in0=in0[:], in1=in1[:], op=op)
                return o

            def ts(in0, s1, op0, s2=None, op1=None):
                o = col()
                nc.vector.tensor_scalar(
                    out=o[:], in0=in0[:], scalar1=s1, scalar2=s2,
                    op0=op0, op1=op1 if op1 is not None else AL.bypass,
                )
                return o

            def stt(in0, s, in1, op0, op1):
                o = col()
                nc.vector.scalar_tensor_tensor(
                    out=o[:], in0=in0[:], scalar=s, in1=in1[:], op0=op0, op1=op1
                )
                return o

            def act(in0, f, scale=1.0):
                o = col()
                nc.scalar.activation(o[:], in0[:], f, scale=scale)
                return o

            def recip(in0):
                o = col()
                nc.vector.reciprocal(o[:], in0[:])
                return o

            def artanh2(z):
                """2*artanh(z) for z in [0, 1)."""
                r1 = ts(z, 1.0, AL.subtract, -1.0, AL.mult)  # (z-1)*-1 = 1-z
                rc = recip(r1)
                q = stt(z, 1.0, rc, AL.add, AL.mult)          # (1+z)/(1-z)
                return act(q, AF.Ln)

            def batched1(xn2, mxn2, mxhb, y2):
                """alpha, beta with t = alpha*mx + beta*hb."""
                xn = act(xn2, AF.Sqrt)
                zc = ts(xn, MIN_NORM, AL.max)
                z = ts(zc, CLIP, AL.min)
                u2 = artanh2(z)
                mxn = act(mxn2, AF.Sqrt)
                mc = ts(mxn, MIN_NORM, AL.max)
                t1 = tt(mc, recip(zc), AL.mult)
                t2 = tt(t1, u2, AL.mult)
                th = act(t2, AF.Tanh, scale=0.5)            # tanh(mxn/xn*artanh(xn))
                scl = tt(th, recip(mc), AL.mult)            # mobius_matvec scale
                rnc = ts(th, MIN_NORM, AL.max)              # ||res|| = th
                f = ts(recip(rnc), MAXN, AL.mult, 1.0, AL.min)
                s = tt(scl, f, AL.mult)                     # res_p = s*mx
                e_ = tt(th, f, AL.mult)                     # ||res_p||
                x2 = tt(e_, e_, AL.mult)
                xy = tt(s, mxhb, AL.mult)
                p_ = ts(xy, 2.0, AL.mult, 1.0, AL.add)      # 1+2xy
                a_c = ts(p_, y2, AL.add)
                den = stt(x2, y2, p_, AL.mult, AL.add)
                rden = recip(ts(den, MIN_NORM, AL.max))
                s1v = tt(tt(a_c, rden, AL.mult), s, AL.mult)
                b_c = ts(x2, 1.0, AL.subtract, -1.0, AL.mult)  # 1-x2
                s2v = tt(b_c, rden, AL.mult)
                # ||h||^2 analytic:  h = s1*mx + s2*hb
                c1 = tt(s1v, mxn2, AL.mult)
                c2 = tt(s2v, mxhb, AL.mult)
                c3 = stt(c2, 2.0, c1, AL.mult, AL.add)
                c4 = tt(s1v, c3, AL.mult)
                c5 = act(s2v, AF.Square, scale=float(np.sqrt(y2)))
                hn2 = tt(c4, c5, AL.add)
                hn = act(hn2, AF.Sqrt)
                hnc = ts(hn, MIN_NORM, AL.max)
                f2 = ts(recip(hnc), MAXN, AL.mult, 1.0, AL.min)
                pn = tt(hn, f2, AL.mult)
                pnc = ts(pn, MIN_NORM, AL.max)              # <= MAXN so no upper clip
                u2b = artanh2(pnc)
                t4 = tt(u2b, recip(pnc), AL.mult)           # 2*artanh(pn)/pn
                t5 = tt(t4, f2, AL.mult)
                alpha = ts(tt(t5, s1v, AL.mult), 0.5, AL.mult)
                beta = ts(tt(t5, s2v, AL.mult), 0.5, AL.mult)
                return alpha, beta

            def batched2(an2, aghb, y2p):
                """g1, g2 with out = g1*agg + g2*hbp."""
                an = act(an2, AF.Sqrt)
                anc = ts(an, MIN_NORM, AL.max)
                th2 = act(an, AF.Tanh)                      # ||expmap0(agg)||
                esc = tt(th2, recip(anc), AL.mult)
                thc = ts(th2, MIN_NORM, AL.max)
                f3 = ts(recip(thc), MAXN, AL.mult, 1.0, AL.min)
                s_e = tt(esc, f3, AL.mult)
                e2 = tt(th2, f3, AL.mult)
                x2e = tt(e2, e2, AL.mult)
                xye = tt(s_e, aghb, AL.mult)
                p2 = ts(xye, 2.0, AL.mult, 1.0, AL.add)
                a2c = ts(p2, y2p, AL.add)
                den2 = stt(x2e, y2p, p2, AL.mult, AL.add)
                rden2 = recip(ts(den2, MIN_NORM, AL.max))
                u1 = tt(tt(a2c, rden2, AL.mult), s_e, AL.mult)
                b2c = ts(x2e, 1.0, AL.subtract, -1.0, AL.mult)
                u2c = tt(b2c, rden2, AL.mult)
                d1 = tt(u1, an2, AL.mult)
                d2 = tt(u2c, aghb, AL.mult)
                d3 = stt(d2, 2.0, d1, AL.mult, AL.add)
                d4 = tt(u1, d3, AL.mult)
                d5 = act(u2c, AF.Square, scale=float(np.sqrt(y2p)))
                on2 = tt(d4, d5, AL.add)
                on = act(on2, AF.Sqrt)
                onc = ts(on, MIN_NORM, AL.max)
                f5 = ts(recip(onc), MAXN, AL.mult, 1.0, AL.min)
                g1 = tt(f5, u1, AL.mult)
                g2 = tt(f5, u2c, AL.mult)
                return g1, g2

            # per-chunk round/block boundary flags from the seg-major stream
            nch_of = {}
            for (b, s_, kk) in sched:
                nch_of[(b, s_)] = max(nch_of.get((b, s_), 0), kk + 1)
            rounds_of = {}
            for (b, s_, kk) in sched:
                if kk == 0:
                    rounds_of.setdefault(b, []).append(s_)

            for l in range(2):
                # ---------------- stage 1: dense ----------------
                xn2 = colp.tile([P, NT], F32, name="xn2", tag="stats", bufs=10)
                mxn2 = colp.tile([P, NT], F32, name="xn2", tag="stats", bufs=10)
                mxhb = colp.tile([P, NT], F32, name="mxhb", tag="stats", bufs=10)
                for b in range(NT):
                    scr = spool.tile([P, D], F32, tag="scr", name="scr")
                    nc.scalar.activation(
                        scr[:], a_t(b), AF.Square, accum_out=xn2[:, b : b + 1]
                    )
                    pt = psT.tile([P, D], F32, space="PSUM", tag="psT", name="psT")
                    nc.tensor.transpose(out=pt[:], in_=a_t(b), identity=ident[:])
                    at_sb = atp.tile([P, D], F32, tag="aT", name="aT")
                    nc.vector.tensor_copy(at_sb[:], pt[:])
                    pm = psM.tile([P, D], F32, space="PSUM", tag="psM", name="psM")
                    nc.tensor.matmul(
                        out=pm[:], lhsT=at_sb[:], rhs=w_sb[l][:], start=True, stop=True
                    )
                    scr2 = spool.tile([P, D], F32, tag="scr", name="scr")
                    nc.scalar.activation(
                        scr2[:], pm[:], AF.Square, accum_out=mxn2[:, b : b + 1]
                    )
                    scr3 = spool.tile([P, D], F32, tag="scr", name="scr")
                    nc.vector.scalar_tensor_tensor(
                        out=scr3[:], in0=pm[:], scalar=1.0, in1=hbl_sb[l][:],
                        op0=AL.mult, op1=AL.mult, accum_out=mxhb[:, b : b + 1],
                    )
                    nc.vector.tensor_copy(a_t(b), pm[:])  # mx overwrites a

                alpha, beta = batched1(xn2, mxn2, mxhb, y2_lin[l])

                for b in range(NT):
                    u = uvp.tile([P, D], F32, tag="uv", name="uv")
                    nc.scalar.activation(
                        u[:], hbl_sb[l][:], AF.Copy, scale=beta[:, b : b + 1]
                    )
                    nc.vector.scalar_tensor_tensor(
                        out=t_t(b), in0=a_t(b), scalar=alpha[:, b : b + 1],
                        in1=u[:], op0=AL.mult, op1=AL.add,
                    )

                if l == 0 and DEBUG_STAGE == "t1":
                    for b in range(NT):
                        dbg = uvp.tile([P, D], F32, tag="uv", name="uv")
                        nc.vector.tensor_copy(dbg[:], t_t(b))
                        nc.sync.dma_start(outp[:, b * D : (b + 1) * D], dbg[:])
                    break
                if l == 0 and DEBUG_STAGE == "mx1":
                    for b in range(NT):
                        nc.sync.dma_start(outp[:, b * D : (b + 1) * D], a_t(b))
                    break
                # t -> DRAM (row-major) and all-gather
                nc.sync.dma_start(
                    tsrc[l][:].rearrange("(t p) d -> p t d", p=P),
                    tbf[:].rearrange("p (t d) -> p t d", d=D),
                )
                nc.gpsimd.collective_compute(
                    "AllGather", AL.bypass, replica_groups=rg,
                    ins=[tsrc[l][:]], outs=[tful[l][:]],
                )

                # ---------------- stage 2: gather + scatter ----------------
                an2 = colp.tile([P, NT], F32, name="an2", tag="stats", bufs=10)
                aghb = colp.tile([P, NT], F32, name="aghb", tag="stats", bufs=10)
                ps_cur = None
                c0 = 0
                while c0 < TC:
                    c1 = c0 + 1
                    while (
                        c1 < TC
                        and c1 - c0 < GCHUNKS
                        and sched[c1][1] == sched[c0][1]
                    ):
                        c1 += 1
                    R = c1 - c0
                    s_ = sched[c0][1]
                    gbuf = gpool.tile([P, GCHUNKS * D], BF16, tag="gat", name="gat")
                    nc.gpsimd.dma_gather(
                        out_ap=gbuf[:, : R * D].rearrange("p (c d) -> p c d", d=D),
                        in_ap=tful[l][s_ * SEGR : (s_ + 1) * SEGR, :],
                        idxs_ap=idxg_sb[:, c0 * 8 : c1 * 8],
                        num_idxs=R * P,
                        num_idxs_reg=R * P,
                        elem_size=D,
                        single_packet=False,
                    )
                    ohb = ohpool.tile([P, GCHUNKS * D], BF16, tag="oh", name="oh")
                    io3 = iota_sb[:].rearrange("p (a d) -> p a d", a=1)
                    do3 = dstg_sb[:, c0:c1].rearrange("p (c a) -> p c a", a=1)
                    i_b, d_b = bass.broadcast_tensor_aps(io3, do3)
                    nc.vector.tensor_tensor(
                        out=ohb[:, : R * D].rearrange("p (c d) -> p c d", d=D),
                        in0=i_b, in1=d_b, op=AL.is_equal,
                    )
                    for c in range(c0, c1):
                        b, s_c, kk = sched[c]
                        cc = c - c0
                        first = kk == 0
                        last = kk == nch_of[(b, s_c)] - 1
                        if first:
                            ps_cur = psA.tile(
                                [P, D], F32, space="PSUM", tag="psA", name="psA"
                            )
                        nc.tensor.matmul(
                            out=ps_cur[:],
                            lhsT=ohb[:, cc * D : (cc + 1) * D],
                            rhs=gbuf[:, cc * D : (cc + 1) * D],
                            start=first, stop=last,
                        )
                        if last:
                            rounds = rounds_of[b]
                            prev = t_t(b) if s_c == rounds[0] else a_t(b)
                            nc.vector.tensor_tensor(
                                out=a_t(b), in0=ps_cur[:], in1=prev, op=AL.add
                            )
                            if s_c == rounds[-1]:
                                s4 = spool.tile([P, D], F32, tag="scr", name="scr")
                                nc.scalar.activation(
                                    s4[:], a_t(b), AF.Square,
                                    accum_out=an2[:, b : b + 1],
                                )
                                s5 = spool.tile([P, D], F32, tag="scr", name="scr")
                                nc.vector.scalar_tensor_tensor(
                                    out=s5[:], in0=a_t(b), scalar=1.0,
                                    in1=hbp_sb[l][:], op0=AL.mult, op1=AL.mult,
                                    accum_out=aghb[:, b : b + 1],
                                )
                    c0 = c1

                if l == 0 and DEBUG_STAGE == "agg1":
                    nc.sync.dma_start(outp[:], a_buf[:])
                    break
                g1c, g2c = batched2(an2, aghb, y2_post[l])
                for b in range(NT):
                    v = uvp.tile([P, D], F32, tag="uv", name="uv")
                    nc.scalar.activation(
                        v[:], hbp_sb[l][:], AF.Copy, scale=g2c[:, b : b + 1]
                    )
                    nc.vector.scalar_tensor_tensor(
                        out=a_t(b), in0=a_t(b), scalar=g1c[:, b : b + 1],
                        in1=v[:], op0=AL.mult, op1=AL.add,
                    )

            nc.sync.dma_start(outp[:], a_buf[:])

    nc.finalize()
    return nc


# ----------------------------------------------------------------------------
# host wrapper
# ----------------------------------------------------------------------------
_cache = {}


def _prepare(x, edge_index, W1, blin1, b1, W2, blin2, b2):
    src = np.asarray(edge_index[0])
    dst = np.asarray(edge_index[1])
    shard = dst // NL
    ldst = dst - shard * NL
    blk = ldst // P
    off = (ldst - blk * P).astype(np.float32)
    srcp = ((src // NL) * NLP + (src - (src // NL) * NL)).astype(np.int32)

    seg = srcp // SEGR
    lidx = (srcp - seg * SEGR).astype(np.int16)

    # bucket edges by (core, block, segment)
    key = (shard * NT + blk) * NSEG + seg
    counts = np.bincount(key, minlength=C * NT * NSEG).reshape(C, NT, NSEG)
    nch = np.ceil(counts.max(axis=0) / P).astype(np.int64)  # [NT, NSEG]
    # seg-major chunk stream: for s: for b: nch[b, s] chunks
    sched = []
    for s_ in range(NSEG):
        for b in range(NT):
            for kk in range(int(nch[b, s_])):
                sched.append((b, s_, kk))
    TC = len(sched)
    cstart = np.zeros((NT, NSEG), np.int64)  # first chunk id of (b, s)
    for ci, (b, s_, kk) in enumerate(sched):
        if kk == 0:
            cstart[b, s_] = ci

    order = np.lexsort((np.arange(E), key))
    bounds = np.concatenate([[0], np.cumsum(counts.reshape(-1))])
    lane = np.zeros((C, TC, P), np.int16)       # per-chunk lane -> seg-local idx
    dstg = np.full((C, P, TC), 255.0, np.float32)
    for c in range(C):
        for b in range(NT):
            for s_ in range(NSEG):
                k = (c * NT + b) * NSEG + s_
                ee = order[bounds[k] : bounds[k + 1]]
                n = len(ee)
                nchunk = int(nch[b, s_])
                if nchunk == 0:
                    continue
                nslot = nchunk * P
                l_pad = np.zeros(nslot, np.int16)
                d_pad = np.full(nslot, 255.0, np.float32)
                l_pad[:n] = lidx[ee]
                d_pad[:n] = off[ee]
                t0 = cstart[b, s_]
                lane[c, t0 : t0 + nchunk] = l_pad.reshape(nchunk, P)
                dstg[c, :, t0 : t0 + nchunk] = d_pad.reshape(nchunk, P).T
    # dma_gather idx wrap: call-relative pos i -> [i % 16, i // 16]; per chunk
    # c the columns are [c*8, (c+1)*8) with row p % 16, col-offset p // 16.
    idxg = lane.reshape(C, TC * 8, 16).transpose(0, 2, 1)  # [C, 16, TC*8]
    idxg = np.tile(idxg, (1, 8, 1))  # replicate across the 8 16-row groups

    # x -> padded, partition-major [C, P, NT*D]
    xpad = np.zeros((C, NT, P, D), np.float32)
    xr = np.asarray(x).reshape(C, NL, D)
    xpad.reshape(C, NLP, D)[:, :NL] = xr
    xp = xpad.transpose(0, 2, 1, 3).reshape(C, P, NT * D)

    hb_l1 = _np_hb(np.asarray(blin1))
    hb_p1 = _np_hb(np.asarray(b1))
    hb_l2 = _np_hb(np.asarray(blin2))
    hb_p2 = _np_hb(np.asarray(b2))
    y2_lin = [float(np.sum(hb_l1 * hb_l1)), float(np.sum(hb_l2 * hb_l2))]
    y2_post = [float(np.sum(hb_p1 * hb_p1)), float(np.sum(hb_p2 * hb_p2))]

    def bf16(a):
        import ml_dtypes

        return a.astype(ml_dtypes.bfloat16)

    iota = bf16(np.tile(np.arange(D, dtype=np.float32)[None, :], (P, 1)))

    in_maps = []
    for c in range(C):
        m = {
            "xp": xp[c],
            "w1t": np.asarray(W1).T.copy(),
            "w2t": np.asarray(W2).T.copy(),
            "hbl1": np.tile(hb_l1[None, :], (P, 1)),
            "hbl2": np.tile(hb_l2[None, :], (P, 1)),
            "hbp1": np.tile(hb_p1[None, :], (P, 1)),
            "hbp2": np.tile(hb_p2[None, :], (P, 1)),
            "iota": iota,
            "idxg": idxg[c],
            "dstg": bf16(dstg[c]),
        }
        in_maps.append(m)
    return in_maps, sched, y2_lin, y2_post


def kernel(x, edge_index, W1, blin1, b1, W2, blin2, b2, trace=False):
    in_maps, sched, y2_lin, y2_post = _prepare(
        x, edge_index, W1, blin1, b1, W2, blin2, b2
    )
    key = (tuple(sched), tuple(y2_lin), tuple(y2_post))
    if key not in _cache:
        _cache[key] = build_kernel(sched, y2_lin, y2_post)
    nc = _cache[key]
    res = run_bass_kernel_spmd(nc, in_maps, core_ids=list(range(C)), trace=trace)
    outs = res.results
    full = np.empty((N, D), np.float32)
    for c in range(C):
        o = np.asarray(outs[c]["outp"]).reshape(P, NT, D).transpose(1, 0, 2)
        full[c * NL : (c + 1) * NL] = o.reshape(NLP, D)[:NL]
    kernel._last_exec_ns = res.exec_time_ns
    kernel._last_res = res
    return full

